# revision 1
# baseline (speedup 1.0000x reference)
"""CrossViewEnhancement Trainium2 kernel (8-core batch-parallel SPMD).

Reference computation (per batch element):
    q = avgpool2(conv1x1(bev_x, qw, qb))                   [C8, 64, 64]
    k = avgpool2(conv1x1(front_x, kw, kb)).mean(h)         [C8, 64]
    v = avgpool2(conv3x3(front_x, vw, vb, pad=1))          [C, 64, 64]
    e = einsum(k, q); L2-normalize over h per column       [64, 64]
    T = e * v.sum(h); nn-upsample x2                       [C, 128, 128]
    out = cat(bev[:16], conv3x3(cat(bev[16:], T), fw, fb))

Algebraic restructuring (validated exactly vs reference in fp32 numpy):
  * v only enters via vsum = v.sum(h): the 3x3 conv collapses to 1-D 3-tap
    convs over colsum(front_x) with row-0 / row-127 edge corrections
    (19.3 GFLOP -> 0.15 GFLOP).
  * k only needs colsum; q's 1x1 conv commutes with pooling - the 2x2
    pooling is folded into strided matmul rhs access patterns (4
    accumulating matmuls) reading the bev halo bands.
  * conv3x3 over the x2-nearest-upsampled T decomposes into 4 output-parity
    phases, each a 2x2-tap conv on half-res Tp with parity-summed weights.
  * the dense bev-channel part of the fusion conv is 9 shifted matmuls
    over zero-padded bf16 halo bands covering all 256 bev channels
    (fusion weights zero-padded over channels 0..15 - identical PE cost).

Host-side prep is constant folding + dtype/layout only: weight transposes
and parity-sums in numpy, plus bf16 copies of the two activation inputs
(they are consumed in bf16 by the tensor engine anyway).

Toolchain constraints honored throughout: the DMA DIRECT2D instruction
encodes at most ONE semaphore wait, so every DMA writes a FRESH tile
(no slot reuse); recycled pool address windows are separated by
all-engine barriers; multi-dependency PSUM->SBUF copies run on the
vector engine (larger wait budget than ACT).
"""

import numpy as np
import ml_dtypes

import concourse.bass as bass
import concourse.mybir as mybir
from concourse.alu_op_type import AluOpType
from concourse.tile import TileContext
from concourse.bass_utils import run_bass_kernel_spmd

F32 = mybir.dt.float32
BF16 = mybir.dt.bfloat16
AX = mybir.AxisListType
AF = mybir.ActivationFunctionType

B, C, H, W = 8, 256, 128, 128
C8 = 32
CO = C - 16          # 240
HP = H // 2          # 64
WP = W // 2          # 64
NCORES = 8
HB = 130             # halo row length (128 + 2)
TPW = WP + 2         # 66
NB = 8               # output row groups of 16
MBLK = [(0, 128), (128, 112)]    # out-channel blocks of the 240
KBLK = [(0, 128), (128, 128)]    # input-channel blocks of 256


def _tap_groups(par):
    return [[0], [1, 2]] if par == 0 else [[0, 1], [2]]


def _tap_offsets(par):
    return [-1, 0] if par == 0 else [0, 1]


def _prep_inputs(inputs):
    bf = ml_dtypes.bfloat16
    qw = np.asarray(inputs["qw"], np.float32)[:, :, 0, 0]
    kw = np.asarray(inputs["kw"], np.float32)[:, :, 0, 0]
    vw = np.asarray(inputs["vw"], np.float32)
    vb = np.asarray(inputs["vb"], np.float32)
    qb = np.asarray(inputs["qb"], np.float32)
    kb = np.asarray(inputs["kb"], np.float32)
    fw = np.asarray(inputs["fw"], np.float32)
    fb = np.asarray(inputs["fb"], np.float32)

    W2 = vw.sum(axis=2)                               # [C, C, 3]
    WV = np.zeros((9, C, C), np.float32)              # [src*3+dx, cin, cout]
    for dx in range(3):
        WV[0 * 3 + dx] = W2[:, :, dx].T
        WV[1 * 3 + dx] = -vw[:, :, 0, dx].T           # -row127 correction
        WV[2 * 3 + dx] = -vw[:, :, 2, dx].T           # -row0 correction
    Wq = qw.T * 0.25                                  # [C, C8]
    Wk = kw.T / 256.0                                 # [C, C8]
    # Part A weights over all 256 bev channels, rows 0..15 zero.
    WA = np.zeros((9, C, CO), np.float32)             # [dy*3+dx, cin, o]
    fwA = np.transpose(fw[:, :CO], (2, 3, 1, 0))      # [dy, dx, cin240, o]
    WA[:, 16:, :] = fwA.reshape(9, CO, CO)
    fwB = fw[:, CO:]                                  # [240, 256, 3, 3]
    WB = np.zeros((16, C, CO), np.float32)            # [((ph*2+pw)*2+i)*2+j]
    for ph in range(2):
        for pw in range(2):
            for i, dys in enumerate(_tap_groups(ph)):
                for j, dxs in enumerate(_tap_groups(pw)):
                    acc = np.zeros((C, CO), np.float32)
                    for dy in dys:
                        for dx in dxs:
                            acc += fwB[:, :, dy, dx].T
                    WB[((ph * 2 + pw) * 2 + i) * 2 + j] = acc
    front = np.asarray(inputs["front_x"], np.float32)
    bev = np.asarray(inputs["bev_x"], np.float32)
    shared = {
        "WV": WV.astype(bf),
        "Wq": Wq.astype(bf),
        "Wk": Wk.astype(bf),
        "WA": WA.astype(bf),
        "WB": WB.astype(bf),
        "vbias": (64.0 * vb).astype(np.float32),
        "qb": qb.astype(np.float32),
        "kb": kb.astype(np.float32),
        "fb": fb.astype(np.float32),
        "ones": np.ones((C8, 128), bf),
    }
    in_maps = []
    for b in range(NCORES):
        m = {
            "front_b": np.ascontiguousarray(front[b].astype(bf)),
            "bev_b": np.ascontiguousarray(bev[b].astype(bf)),
            "bev16": np.ascontiguousarray(bev[b, :16]),
        }
        m.update(shared)
        in_maps.append(m)
    return in_maps


def _build_module():
    nc = bass.Bass()
    fx_d = nc.dram_tensor("front_b", [C, H, W], BF16, kind="ExternalInput")
    bx_d = nc.dram_tensor("bev_b", [C, H, W], BF16, kind="ExternalInput")
    b16_d = nc.dram_tensor("bev16", [16, H, W], F32, kind="ExternalInput")
    WV_d = nc.dram_tensor("WV", [9, C, C], BF16, kind="ExternalInput")
    Wq_d = nc.dram_tensor("Wq", [C, C8], BF16, kind="ExternalInput")
    Wk_d = nc.dram_tensor("Wk", [C, C8], BF16, kind="ExternalInput")
    WA_d = nc.dram_tensor("WA", [9, C, CO], BF16, kind="ExternalInput")
    WB_d = nc.dram_tensor("WB", [16, C, CO], BF16, kind="ExternalInput")
    vbias_d = nc.dram_tensor("vbias", [C], F32, kind="ExternalInput")
    qb_d = nc.dram_tensor("qb", [C8], F32, kind="ExternalInput")
    kb_d = nc.dram_tensor("kb", [C8], F32, kind="ExternalInput")
    fb_d = nc.dram_tensor("fb", [CO], F32, kind="ExternalInput")
    ones_d = nc.dram_tensor("ones", [C8, 128], BF16, kind="ExternalInput")
    out_d = nc.dram_tensor("out", [C, H, W], F32, kind="ExternalOutput")

    with TileContext(nc) as tc:
        # out[:16] = bev[:16] straight through, DRAM->DRAM
        nc.sync.dma_start(out=out_d[0:16], in_=b16_d[:])

        with (
            tc.tile_pool(name="stage", bufs=1) as stp,
            tc.tile_pool(name="early", bufs=1) as epp,
            tc.tile_pool(name="bands", bufs=1) as bandp,
        ):
            # --------- bev halo bands (bf16, all 256 channels) -----------
            # band[g][blk]: padded rows 16g..16g+17 (h = 16g-1..16g+16)
            bands = [[None, None] for _ in range(NB)]
            for g in range(NB):
                for bl, (c0, cs) in enumerate(KBLK):
                    bt = bandp.tile([cs, 18 * HB], BF16,
                                    name=f"band_{g}_{bl}",
                                    tag=f"band_{g}_{bl}")
                    bands[g][bl] = bt
                    v = bt[:].rearrange("p (r c) -> p r c", r=18)
                    nc.gpsimd.memset(v[:, :, 0:1], 0.0)
                    nc.gpsimd.memset(v[:, :, HB - 1:HB], 0.0)
                    h_lo, r0, nrows = 16 * g - 1, 0, 18
                    if g == 0:
                        nc.gpsimd.memset(v[:, 0:1, :], 0.0)
                        h_lo, r0, nrows = 0, 1, 17
                    if g == NB - 1:
                        nc.gpsimd.memset(v[:, 17:18, :], 0.0)
                        nrows -= 1
                    nc.sync.dma_start(
                        out=v[:, r0:r0 + nrows, 1:1 + W],
                        in_=bx_d[c0:c0 + cs, h_lo:h_lo + nrows, :])

            # --------- small weights + colsum inputs (early pool) --------
            WV_t = []
            for sd in range(9):
                row = []
                for kb_i, (k0, ks) in enumerate(KBLK):
                    t = epp.tile([ks, C], BF16, name=f"WV_{sd}_{kb_i}",
                                 tag=f"WV_{sd}_{kb_i}")
                    nc.sync.dma_start(out=t[:], in_=WV_d[sd, k0:k0 + ks, :])
                    row.append(t)
                WV_t.append(row)
            Wq_t, Wk_t = [], []
            for kb_i, (k0, ks) in enumerate(KBLK):
                tq = epp.tile([ks, C8], BF16, name=f"Wq_{kb_i}",
                              tag=f"Wq_{kb_i}")
                nc.sync.dma_start(out=tq[:], in_=Wq_d[k0:k0 + ks, :])
                Wq_t.append(tq)
                tk = epp.tile([ks, C8], BF16, name=f"Wk_{kb_i}",
                              tag=f"Wk_{kb_i}")
                nc.sync.dma_start(out=tk[:], in_=Wk_d[k0:k0 + ks, :])
                Wk_t.append(tk)
            ones_t = epp.tile([C8, 128], BF16, name="ones_t", tag="ones_t")
            nc.sync.dma_start(out=ones_t[:], in_=ones_d[:])
            vbias_t = []
            for bl in range(2):
                t = epp.tile([128, 1], F32, name=f"vbias_{bl}",
                             tag=f"vbias_{bl}")
                nc.sync.dma_start(
                    out=t[:], in_=vbias_d[bl * 128:(bl + 1) * 128].unsqueeze(1))
                vbias_t.append(t)
            qb_t = epp.tile([C8, 1], F32, name="qb_t", tag="qb_t")
            nc.sync.dma_start(out=qb_t[:], in_=qb_d[:].unsqueeze(1))
            kb_t = epp.tile([C8, 1], F32, name="kb_t", tag="kb_t")
            nc.sync.dma_start(out=kb_t[:], in_=kb_d[:].unsqueeze(1))

            # ---- stage 1: colsum + edge rows of front (scoped pool) ----
            X3b, P2b = [], []
            with tc.tile_pool(name="fstr", bufs=1) as fp_:
                for bl in range(2):
                    c0 = bl * 128
                    csum = epp.tile([128, W], F32, name=f"colsum_{bl}",
                                    tag=f"colsum_{bl}")
                    for half in range(2):
                        ch = fp_.tile([128, 64, W], BF16,
                                      name=f"fch_{bl}_{half}",
                                      tag=f"fch_{bl}_{half}")
                        nc.sync.dma_start(
                            out=ch[:],
                            in_=fx_d[c0:c0 + 128,
                                     half * 64:(half + 1) * 64, :])
                        if half == 0:
                            nc.vector.tensor_reduce(
                                out=csum[:],
                                in_=ch[:].rearrange("p r c -> p c r"),
                                axis=AX.X, op=AluOpType.add)
                        else:
                            part = fp_.tile([128, W], F32,
                                            name=f"fpart_{bl}",
                                            tag=f"fpart_{bl}")
                            nc.vector.tensor_reduce(
                                out=part[:],
                                in_=ch[:].rearrange("p r c -> p c r"),
                                axis=AX.X, op=AluOpType.add)
                            nc.vector.tensor_add(
                                out=csum[:], in0=csum[:], in1=part[:])
                    r0t = epp.tile([128, W], BF16, name=f"r0_{bl}",
                                   tag=f"r0_{bl}")
                    rLt = epp.tile([128, W], BF16, name=f"rL_{bl}",
                                   tag=f"rL_{bl}")
                    nc.sync.dma_start(out=r0t[:], in_=fx_d[c0:c0 + 128, 0, :])
                    nc.sync.dma_start(out=rLt[:],
                                      in_=fx_d[c0:c0 + 128, H - 1, :])
                    x3v = epp.tile([128, 3 * HB], BF16, name=f"x3_{bl}",
                                   tag=f"x3_{bl}")
                    xv = x3v[:].rearrange("p (s c) -> p s c", s=3)
                    nc.gpsimd.memset(xv[:, :, 0:1], 0.0)
                    nc.gpsimd.memset(xv[:, :, HB - 1:HB], 0.0)
                    nc.vector.tensor_copy(out=xv[:, 0, 1:1 + W], in_=csum[:])
                    nc.vector.tensor_copy(out=xv[:, 1, 1:1 + W], in_=rLt[:])
                    nc.vector.tensor_copy(out=xv[:, 2, 1:1 + W], in_=r0t[:])
                    X3b.append(x3v)
                    p2 = epp.tile([128, WP], BF16, name=f"p2_{bl}",
                                  tag=f"p2_{bl}")
                    cs3 = csum[:].rearrange("p (w two) -> p w two", two=2)
                    nc.vector.tensor_tensor(
                        out=p2[:], in0=cs3[:, :, 0], in1=cs3[:, :, 1],
                        op=AluOpType.add)
                    P2b.append(p2)

            tc.strict_bb_all_engine_barrier()

            # --------- heavy weights (recycle the colsum window) ---------
            with tc.tile_pool(name="wpers", bufs=1) as wpp:
                WA_t = []
                for sd in range(9):
                    row = []
                    for kb_i, (k0, ks) in enumerate(KBLK):
                        t = wpp.tile([ks, CO], BF16, name=f"WA_{sd}_{kb_i}",
                                     tag=f"WA_{sd}_{kb_i}")
                        nc.sync.dma_start(out=t[:], in_=WA_d[sd, k0:k0 + ks, :])
                        row.append(t)
                    WA_t.append(row)
                WB_t = []
                for cc in range(16):
                    row = []
                    for kb_i, (k0, ks) in enumerate(KBLK):
                        t = wpp.tile([ks, CO], BF16, name=f"WB_{cc}_{kb_i}",
                                     tag=f"WB_{cc}_{kb_i}")
                        nc.sync.dma_start(out=t[:], in_=WB_d[cc, k0:k0 + ks, :])
                        row.append(t)
                    WB_t.append(row)
                fb_t = []
                for mb_i, (m0, ms) in enumerate(MBLK):
                    t = wpp.tile([ms, 1], F32, name=f"fb_{mb_i}",
                                 tag=f"fb_{mb_i}")
                    nc.sync.dma_start(out=t[:],
                                      in_=fb_d[m0:m0 + ms].unsqueeze(1))
                    fb_t.append(t)

                # ======== prefix compute: vsum / k / q / e / Tp ==========
                with (
                    tc.tile_pool(name="pref", bufs=1) as prp,
                    tc.tile_pool(name="psp", bufs=2, space="PSUM") as psp,
                ):
                    # ---- stage 2: S -> vsum ----
                    vsum_t = []
                    for mb in range(2):
                        ps = psp.tile([128, W], F32, name="psS", tag="psS")
                        first = True
                        for sd in range(9):
                            src, dx = divmod(sd, 3)
                            for kb_i in range(2):
                                xv = X3b[kb_i][:].rearrange(
                                    "p (s c) -> p s c", s=3)
                                nc.tensor.matmul(
                                    ps[:],
                                    WV_t[sd][kb_i][:, mb * 128:(mb + 1) * 128],
                                    xv[:, src, dx:dx + W],
                                    start=first, stop=(sd == 8 and kb_i == 1))
                                first = False
                        ssb = prp.tile([128, W], F32, name=f"ssb_{mb}",
                                       tag=f"ssb_{mb}")
                        nc.scalar.activation(out=ssb[:], in_=ps[:],
                                             func=AF.Copy, scale=0.25)
                        se = ssb[:].rearrange("p (w two) -> p w two", two=2)
                        vs = prp.tile([128, WP], F32, name=f"vsum_{mb}",
                                      tag=f"vsum_{mb}")
                        nc.vector.scalar_tensor_tensor(
                            out=vs[:], in0=se[:, :, 0],
                            scalar=vbias_t[mb][:], in1=se[:, :, 1],
                            op0=AluOpType.add, op1=AluOpType.add)
                        vsum_t.append(vs)

                    # ---- stage 3: k ----
                    psk = psp.tile([C8, WP], F32, name="psK", tag="psK")
                    nc.tensor.matmul(psk[:], Wk_t[0][:], P2b[0][:],
                                     start=True, stop=False)
                    nc.tensor.matmul(psk[:], Wk_t[1][:], P2b[1][:],
                                     start=False, stop=True)
                    k_t = prp.tile([C8, WP], F32, name="k_t", tag="k_t")
                    nc.vector.tensor_scalar_add(out=k_t[:], in0=psk[:],
                                                scalar1=kb_t[:])

                    # ---- stage 4: q (pooling inside strided rhs) + qk ----
                    qk_t = prp.tile([C8, HP * WP], BF16, name="qk_t",
                                    tag="qk_t")
                    qkv = qk_t[:].rearrange("p (h w) -> p h w", w=WP)
                    for g in range(NB):
                        psq = psp.tile([C8, 8 * WP], F32, name="psQ",
                                       tag="psQ")
                        first = True
                        for i in range(2):
                            for j in range(2):
                                for kb_i in range(2):
                                    bv = bands[g][kb_i][:].rearrange(
                                        "p (r c) -> p r c", c=HB)
                                    rhs = bv[:, 1 + i:17 + i:2,
                                             1 + j:129 + j:2]
                                    nc.tensor.matmul(
                                        psq[:], Wq_t[kb_i][:], rhs,
                                        start=first,
                                        stop=(i == 1 and j == 1
                                              and kb_i == 1))
                                    first = False
                        qtmp = prp.tile([C8, 8 * WP], F32, name="qtmp",
                                        tag="qtmp", bufs=2)
                        nc.vector.tensor_scalar_add(out=qtmp[:], in0=psq[:],
                                                    scalar1=qb_t[:])
                        kv = k_t[:].unsqueeze(1).broadcast_to([C8, 8, WP])
                        nc.vector.tensor_tensor(
                            out=qkv[:, g * 8:(g + 1) * 8, :],
                            in0=qtmp[:].rearrange("p (h w) -> p h w", w=WP),
                            in1=kv, op=AluOpType.mult)

                    # ---- stage 5: e (replicated) + column norms ----
                    e_t = prp.tile([128, HP * WP], BF16, name="e_t",
                                   tag="e_t")
                    for chn in range(8):
                        nsl = slice(chn * 512, (chn + 1) * 512)
                        pse = psp.tile([128, 512], F32, name="psE", tag="psE")
                        nc.tensor.matmul(pse[:], ones_t[:], qk_t[:, nsl],
                                         start=True, stop=True)
                        nc.vector.tensor_copy(out=e_t[:, nsl], in_=pse[:])
                    n2 = prp.tile([128, WP], F32, name="n2", tag="n2")
                    for chn in range(8):
                        scr = prp.tile([128, 8 * WP], F32, name="scr",
                                       tag="scr", bufs=2)
                        esl = e_t[:, chn * 8 * WP:(chn + 1) * 8 * WP]
                        nc.vector.tensor_tensor(out=scr[:], in0=esl, in1=esl,
                                                op=AluOpType.mult)
                        part = prp.tile([128, WP], F32, name="npart",
                                        tag="npart", bufs=2)
                        nc.vector.tensor_reduce(
                            out=part[:],
                            in_=scr[:].rearrange("p (h w) -> p w h", w=WP),
                            axis=AX.X, op=AluOpType.add)
                        if chn == 0:
                            nc.vector.tensor_copy(out=n2[:], in_=part[:])
                        else:
                            nc.vector.tensor_add(out=n2[:], in0=n2[:],
                                                 in1=part[:])
                    nrm = prp.tile([128, WP], F32, name="nrm", tag="nrm")
                    nc.scalar.sqrt(out=nrm[:], in_=n2[:])
                    rinv = prp.tile([128, WP], F32, name="rinv", tag="rinv")
                    nc.vector.reciprocal(out=rinv[:], in_=nrm[:])

                    # ---- stage 6: Tp_pad = (vsum * rinv) x e ----
                    tp_t = []
                    for bl in range(2):
                        vs2 = prp.tile([128, WP], F32, name=f"vs2_{bl}",
                                       tag=f"vs2_{bl}")
                        nc.vector.tensor_tensor(
                            out=vs2[:], in0=vsum_t[bl][:], in1=rinv[:],
                            op=AluOpType.mult)
                        tp = stp.tile([128, (HP + 2) * TPW], BF16,
                                      name=f"tp_{bl}", tag=f"tp_{bl}")
                        tp_t.append(tp)
                        tv = tp[:].rearrange("p (r c) -> p r c", c=TPW)
                        nc.gpsimd.memset(tv[:, 0:1, :], 0.0)
                        nc.gpsimd.memset(tv[:, HP + 1:HP + 2, :], 0.0)
                        nc.gpsimd.memset(tv[:, :, 0:1], 0.0)
                        nc.gpsimd.memset(tv[:, :, TPW - 1:TPW], 0.0)
                        ev = e_t[:].rearrange("p (h w) -> p h w", w=WP)
                        v2 = vs2[:].unsqueeze(1).broadcast_to([128, HP, WP])
                        nc.vector.tensor_tensor(
                            out=tv[:, 1:1 + HP, 1:1 + WP], in0=v2, in1=ev,
                            op=AluOpType.mult)

                tc.strict_bb_all_engine_barrier()

                # ========= g-loop: Part B then Part A per row group ======
                with (
                    tc.tile_pool(name="gout", bufs=2) as gop,
                    tc.tile_pool(name="gstg", bufs=1) as gsp,
                    tc.tile_pool(name="psA", bufs=3, space="PSUM") as psa,
                    tc.tile_pool(name="psB", bufs=3, space="PSUM") as psb,
                ):
                    for g in range(NB):
                        stg = []
                        for mb_i, (m0, ms) in enumerate(MBLK):
                            t = gsp.tile([ms, 16 * W], F32, name=f"stg{mb_i}",
                                         tag=f"stg{mb_i}")
                            stg.append(t)
                        # ---- Part B: 4 phases of 2x2-tap matmuls on Tp ----
                        for ph in range(2):
                            ro = _tap_offsets(ph)
                            for pw in range(2):
                                co = _tap_offsets(pw)
                                for mb_i, (m0, ms) in enumerate(MBLK):
                                    pb_ = psb.tile([ms, 8 * WP], F32,
                                                   name="psBt", tag="psBt")
                                    first = True
                                    for i in range(2):
                                        for j in range(2):
                                            cc = ((ph * 2 + pw) * 2 + i) * 2 + j
                                            for kb_i in range(2):
                                                tv = tp_t[kb_i][:].rearrange(
                                                    "p (r c) -> p r c", c=TPW)
                                                rhs = tv[:,
                                                         8 * g + 1 + ro[i]:
                                                         8 * g + 9 + ro[i],
                                                         1 + co[j]:
                                                         1 + co[j] + WP]
                                                nc.tensor.matmul(
                                                    pb_[:],
                                                    WB_t[cc][kb_i][:,
                                                                   m0:m0 + ms],
                                                    rhs,
                                                    start=first,
                                                    stop=(i == 1 and j == 1
                                                          and kb_i == 1))
                                                first = False
                                    sv = stg[mb_i][:].rearrange(
                                        "p (h two w pw2) -> p h two w pw2",
                                        two=2, w=WP, pw2=2)
                                    nc.vector.tensor_scalar_add(
                                        out=sv[:, :, ph, :, pw],
                                        in0=pb_[:].rearrange(
                                            "p (h w) -> p h w", w=WP),
                                        scalar1=fb_t[mb_i][:])
                        # ---- Part A: dense 3x3 conv over bev channels ----
                        for mb_i, (m0, ms) in enumerate(MBLK):
                            ot = gop.tile([ms, 16, W], F32, name=f"ot{mb_i}",
                                          tag=f"ot{mb_i}")
                            for n in range(4):
                                pa_ = psa.tile([ms, 4 * W], F32, name="psAt",
                                               tag="psAt")
                                first = True
                                for dy in range(3):
                                    for dx in range(3):
                                        for kb_i in range(2):
                                            bv = bands[g][kb_i][:].rearrange(
                                                "p (r c) -> p r c", c=HB)
                                            rhs = bv[:,
                                                     4 * n + dy:4 * n + dy + 4,
                                                     dx:dx + W]
                                            nc.tensor.matmul(
                                                pa_[:],
                                                WA_t[dy * 3 + dx][kb_i][
                                                    :, m0:m0 + ms],
                                                rhs,
                                                start=first,
                                                stop=(dy == 2 and dx == 2
                                                      and kb_i == 1))
                                            first = False
                                nc.vector.tensor_add(
                                    out=ot[:, n * 4:(n + 1) * 4, :],
                                    in0=pa_[:].rearrange(
                                        "p (r c) -> p r c", c=W),
                                    in1=stg[mb_i][:,
                                                  n * 4 * W:(n + 1) * 4 * W]
                                    .rearrange("p (r c) -> p r c", c=W))
                            nc.sync.dma_start(
                                out=out_d[16 + m0:16 + m0 + ms,
                                          16 * g:16 * (g + 1), :],
                                in_=ot[:])
    return nc


def _legalize_waits(nc):
    """This toolchain's codegen accepts at most ONE semaphore wait per
    instruction (the TPB `events` field has a single wait slot). Tile's
    wait assignment can attach several. Hoist all but one wait onto
    standalone EventSemaphore instructions placed immediately before the
    owner on the same engine stream - strictly stronger synchronization,
    so always safe."""
    n_split = 0
    for fn in nc.m.functions:
        for bb in fn.blocks:
            out = []
            for ins in bb.instructions:
                si = ins.sync_info
                if si is not None and len(si.on_wait) > 1:
                    extra = list(si.on_wait[:-1])
                    keep = si.on_wait[-1]
                    for idx, wt in enumerate(extra):
                        ev = mybir.InstEventSemaphore(
                            name=f"{ins.name}_hw{idx}",
                            engine=ins.engine,
                            sync_info=mybir.SyncInfo(on_wait=[wt],
                                                     on_update=[]),
                        )
                        out.append(ev)
                        n_split += 1
                    ins.sync_info = mybir.SyncInfo(
                        on_wait=[keep], on_update=list(si.on_update))
                out.append(ins)
            bb.instructions[:] = out
    return n_split


_NC_CACHE = None


def kernel(**inputs):
    global _NC_CACHE
    in_maps = _prep_inputs(inputs)
    if _NC_CACHE is None:
        _NC_CACHE = _build_module()
        _legalize_waits(_NC_CACHE)
    res = run_bass_kernel_spmd(_NC_CACHE, in_maps, list(range(NCORES)))
    out = np.stack([res.results[b]["out"] for b in range(NCORES)], axis=0)
    return out.astype(np.float32)



# revision 9
# speedup vs baseline: 1.4132x; 1.4132x over previous
"""CrossViewEnhancement Trainium2 kernel (8-core batch-parallel SPMD).

Reference computation (per batch element):
    q = avgpool2(conv1x1(bev_x, qw, qb))                   [C8, 64, 64]
    k = avgpool2(conv1x1(front_x, kw, kb)).mean(h)         [C8, 64]
    v = avgpool2(conv3x3(front_x, vw, vb, pad=1))          [C, 64, 64]
    e = einsum(k, q); L2-normalize over h per column       [64, 64]
    T = e * v.sum(h); nn-upsample x2                       [C, 128, 128]
    out = cat(bev[:16], conv3x3(cat(bev[16:], T), fw, fb))

Algebraic restructuring (validated vs reference):
  * v only enters via vsum = v.sum(h): the 3x3 conv collapses to 1-D 3-tap
    convs over colsum(front_x) with row-0 / row-127 edge corrections.
  * k only needs colsum; q's 1x1 conv commutes with pooling - the 2x2
    pooling is folded into strided matmul rhs access patterns.
  * conv3x3 over the x2-nearest-upsampled T decomposes into 4 output-parity
    phases, each a 2x2-tap conv on half-res Tp with parity-summed weights.
  * the dense bev-channel part of the fusion conv is 9 shifted matmuls
    over the resident bev tiles (fusion weights zero-padded over channels
    0..15 - identical PE cost).

Schedule (v1): bev/front live as two [128, H*W] fp16 tiles filled by
16-row chunk DMAs (4KB contiguous per partition -> big DMA packets).
Per group g: Part A (dense conv, only needs bev) runs first and is
evacuated to fp16 staging by the scalar engine (Identity + fb bias);
the prefix (colsum -> vsum/k/q/e/Tp) overlaps underneath on DVE/PE;
Part B accumulates in PSUM and the gpsimd engine combines
psumB + stgA -> f32 staging -> DMA out. Conv halos are handled by
splitting edge-tap matmuls (no zero-padded copies; first matmul of each
accumulation chain is a full-coverage tap).
"""

import numpy as np

import concourse.bass as bass
import concourse.mybir as mybir
from concourse.alu_op_type import AluOpType
from concourse.tile import TileContext
from concourse.bass_utils import run_bass_kernel_spmd

F32 = mybir.dt.float32
F16 = mybir.dt.float16
AX = mybir.AxisListType
AF = mybir.ActivationFunctionType

B, C, H, W = 8, 256, 128, 128
C8 = 32
CO = C - 16          # 240
HP = H // 2          # 64
WP = W // 2          # 64
NCORES = 8
NB = 8               # output row groups of 16
NK = 2               # 128-channel input blocks
MBLK = [(0, 128), (128, 112)]    # out-channel blocks of the 240
TPW = WP + 2         # 66 (w-padded half-res T)


def _tap_groups(par):
    return [[0], [1, 2]] if par == 0 else [[0, 1], [2]]


def _tap_offsets(par):
    return [-1, 0] if par == 0 else [0, 1]


def _prep_inputs(inputs):
    f16 = np.float16
    qw = np.asarray(inputs["qw"], np.float32)[:, :, 0, 0]
    kw = np.asarray(inputs["kw"], np.float32)[:, :, 0, 0]
    vw = np.asarray(inputs["vw"], np.float32)
    vb = np.asarray(inputs["vb"], np.float32)
    qb = np.asarray(inputs["qb"], np.float32)
    kb = np.asarray(inputs["kb"], np.float32)
    fw = np.asarray(inputs["fw"], np.float32)
    fb = np.asarray(inputs["fb"], np.float32)

    # --- vsum weights: [sd = src*3+dx, cin, cout], pre-scaled by 0.25 ---
    W2 = vw.sum(axis=2)                               # [C, C, 3]
    WV = np.zeros((9, C, C), np.float32)
    for dx in range(3):
        WV[0 * 3 + dx] = W2[:, :, dx].T
        WV[1 * 3 + dx] = -vw[:, :, 0, dx].T           # -row127 correction
        WV[2 * 3 + dx] = -vw[:, :, 2, dx].T           # -row0 correction
    WV *= 0.25
    # pack [128p, kb, sd, m]
    WVp = np.transpose(WV.reshape(9, NK, 128, C), (2, 1, 0, 3)).reshape(128, -1)

    Wq = qw.T * 0.25                                  # [C, C8]
    Wk = kw.T / 256.0                                 # [C, C8]
    Wqk = np.stack([Wq.reshape(NK, 128, C8), Wk.reshape(NK, 128, C8)],
                   axis=2)                            # [kb,128,2,C8]
    Wqkp = np.transpose(Wqk, (1, 0, 2, 3)).reshape(128, -1)

    # --- Part A weights over all 256 bev channels, rows 0..15 zero ---
    WA = np.zeros((9, C, CO), np.float32)             # [dy*3+dx, cin, o]
    fwA = np.transpose(fw[:, :CO], (2, 3, 1, 0))      # [dy, dx, cin240, o]
    WA[:, 16:, :] = fwA.reshape(9, CO, CO)
    WAp = np.transpose(WA.reshape(9, NK, 128, CO), (2, 1, 0, 3)).reshape(128, -1)

    # --- Part B parity-summed weights: [cc, cin, o] ---
    fwB = fw[:, CO:]                                  # [240, 256, 3, 3]
    WB = np.zeros((16, C, CO), np.float32)            # [((ph*2+pw)*2+i)*2+j]
    for ph in range(2):
        for pw in range(2):
            for i, dys in enumerate(_tap_groups(ph)):
                for j, dxs in enumerate(_tap_groups(pw)):
                    acc = np.zeros((C, CO), np.float32)
                    for dy in dys:
                        for dx in dxs:
                            acc += fwB[:, :, dy, dx].T
                    WB[((ph * 2 + pw) * 2 + i) * 2 + j] = acc
    WBp = np.transpose(WB.reshape(16, NK, 128, CO), (2, 1, 0, 3)).reshape(128, -1)

    # --- biases packed [128, 6] f32 ---
    biasp = np.zeros((128, 6), np.float32)
    biasp[:, 0] = 64.0 * vb[:128]
    biasp[:, 1] = 64.0 * vb[128:]
    biasp[:, 2] = fb[:128]
    biasp[:112, 3] = fb[128:]
    biasp[:C8, 4] = qb
    biasp[:C8, 5] = kb

    front = np.asarray(inputs["front_x"], np.float32)
    bev = np.asarray(inputs["bev_x"], np.float32)
    shared = {
        "WVp": WVp.astype(f16),
        "Wqkp": Wqkp.astype(f16),
        "WAp": WAp.astype(f16),
        "WBp": WBp.astype(f16),
        "biasp": biasp,
    }
    in_maps = []
    for b in range(NCORES):
        m = {
            "front_b": np.ascontiguousarray(front[b].astype(f16)),
            "bev_b": np.ascontiguousarray(bev[b].astype(f16)),
            "bev16": np.ascontiguousarray(bev[b, :16]),
        }
        m.update(shared)
        in_maps.append(m)
    return in_maps


def _build_module():
    nc = bass.Bass()
    fx_d = nc.dram_tensor("front_b", [C, H, W], F16, kind="ExternalInput")
    bx_d = nc.dram_tensor("bev_b", [C, H, W], F16, kind="ExternalInput")
    b16_d = nc.dram_tensor("bev16", [16, H, W], F32, kind="ExternalInput")
    WV_d = nc.dram_tensor("WVp", [128, NK * 9 * C], F16, kind="ExternalInput")
    Wqk_d = nc.dram_tensor("Wqkp", [128, NK * 2 * C8], F16, kind="ExternalInput")
    WA_d = nc.dram_tensor("WAp", [128, NK * 9 * CO], F16, kind="ExternalInput")
    WB_d = nc.dram_tensor("WBp", [128, NK * 16 * CO], F16, kind="ExternalInput")
    bias_d = nc.dram_tensor("biasp", [128, 6], F32, kind="ExternalInput")
    out_d = nc.dram_tensor("out", [C, H, W], F32, kind="ExternalOutput")

    with TileContext(nc) as tc:
        with (
            tc.tile_pool(name="main", bufs=1) as mp,
            tc.tile_pool(name="stga", bufs=1) as sap,
            tc.tile_pool(name="fstr", bufs=1) as fsp,
            tc.tile_pool(name="pref", bufs=1) as prp,
            tc.tile_pool(name="outp", bufs=1) as gop,
            tc.tile_pool(name="psA", bufs=3, space="PSUM") as psa,
            tc.tile_pool(name="psB", bufs=3, space="PSUM") as psb,
            tc.tile_pool(name="psP", bufs=2, space="PSUM") as psp,
        ):
            # ---------------- persistent tiles -----------------
            WA_t = mp.tile([128, NK * 9 * CO], F16, name="WA_t", tag="WA_t")
            Wqk_t = mp.tile([128, NK * 2 * C8], F16, name="Wqk_t", tag="Wqk_t")
            bias_t = mp.tile([128, 6], F32, name="bias_t", tag="bias_t")
            WV_t = mp.tile([128, NK * 9 * C], F16, name="WV_t", tag="WV_t")
            WB_t = mp.tile([128, NK * 16 * CO], F16, name="WB_t", tag="WB_t")
            bev_t = [mp.tile([128, H * W], F16, name=f"bev_{kb}", tag=f"bev_{kb}")
                     for kb in range(NK)]
            csum = [mp.tile([128, W], F32, name=f"csum_{kb}", tag=f"csum_{kb}")
                    for kb in range(NK)]
            x3 = [mp.tile([128, 3 * (W + 2)], F16, name=f"x3_{kb}",
                          tag=f"x3_{kb}") for kb in range(NK)]
            P2 = [mp.tile([128, WP], F16, name=f"p2_{kb}", tag=f"p2_{kb}")
                  for kb in range(NK)]
            vsum_t = [mp.tile([128, WP], F32, name=f"vsum_{kb}",
                              tag=f"vsum_{kb}") for kb in range(NK)]
            tp_t = [mp.tile([128, HP * TPW], F16, name=f"tp_{kb}",
                            tag=f"tp_{kb}") for kb in range(NK)]
            ones_t = mp.tile([C8, 128], F16, name="ones_t", tag="ones_t")
            k_t = mp.tile([C8, WP], F32, name="k_t", tag="k_t")

            # prefix scratch (scoped pool, reusable address space)
            qk_t = prp.tile([C8, HP * WP], F16, name="qk_t", tag="qk_t")
            e_t = prp.tile([128, HP * WP], F16, name="e_t", tag="e_t")
            n2 = prp.tile([128, WP], F32, name="n2", tag="n2")
            nrm = prp.tile([128, WP], F32, name="nrm", tag="nrm")
            rinv = prp.tile([128, WP], F32, name="rinv", tag="rinv")
            vs2 = [prp.tile([128, WP], F32, name=f"vs2_{kb}", tag=f"vs2_{kb}")
                   for kb in range(NK)]

            bias_ap = {
                "vb": [bias_t[:, 0:1], bias_t[:, 1:2]],
                "fb": [bias_t[:128, 2:3], bias_t[:112, 3:4]],
                "qb": bias_t[:C8, 4:5],
                "kb": bias_t[:C8, 5:6],
            }

            # ---------------- constant init (gpsimd) ------------
            nc.gpsimd.memset(ones_t[:], 1.0)
            for kb in range(NK):
                xv = x3[kb][:].rearrange("p (s c) -> p s c", s=3)
                nc.gpsimd.memset(xv[:, :, 0:1], 0.0)
                nc.gpsimd.memset(xv[:, :, W + 1:W + 2], 0.0)
                tv = tp_t[kb][:].rearrange("p (h w) -> p h w", w=TPW)
                nc.gpsimd.memset(tv[:, :, 0:1], 0.0)
                nc.gpsimd.memset(tv[:, :, TPW - 1:TPW], 0.0)

            # ---------------- input DMAs (sync ring) -------------
            nc.sync.dma_start(out=WA_t[:], in_=WA_d[:])
            nc.sync.dma_start(out=Wqk_t[:], in_=Wqk_d[:])
            nc.sync.dma_start(out=bias_t[:], in_=bias_d[:])
            # bev chunks (16 rows each); front streams through a 4-slot ring
            fch = [None] * (NK * NB)
            for g in range(NB):
                for kb in range(NK):
                    nc.sync.dma_start(
                        out=bev_t[kb][:, g * 16 * W:(g + 1) * 16 * W],
                        in_=bx_d[kb * 128:(kb + 1) * 128,
                                 g * 16:(g + 1) * 16, :])
            nc.sync.dma_start(out=WV_t[:], in_=WV_d[:])
            for ci in range(NK * NB):
                g, kb = divmod(ci, NK)
                t = fsp.tile([128, 16 * W], F16, name=f"fch_{ci}", tag="fch",
                             bufs=2)
                fch[ci] = t
                nc.sync.dma_start(
                    out=t[:],
                    in_=fx_d[kb * 128:(kb + 1) * 128, g * 16:(g + 1) * 16, :])
            nc.sync.dma_start(out=WB_t[:], in_=WB_d[:])
            # out[:16] = bev[:16] straight through, DRAM->DRAM
            nc.sync.dma_start(out=out_d[0:16], in_=b16_d[:])

            # -------- colsum of front (DVE, per streamed chunk) --------
            for ci in range(NK * NB):
                g, kb = divmod(ci, NK)
                cv = fch[ci][:].rearrange("p (h w) -> p w h", w=W)
                if g == 0:
                    nc.vector.tensor_reduce(out=csum[kb][:], in_=cv,
                                            axis=AX.X, op=AluOpType.add)
                    # row 0 of front -> x3 src 2
                    xv = x3[kb][:].rearrange("p (s c) -> p s c", s=3)
                    nc.vector.tensor_copy(
                        out=xv[:, 2, 1:1 + W], in_=fch[ci][:, 0:W])
                else:
                    part = fsp.tile([128, W], F32, name=f"fpart_{ci}",
                                    tag="fpart", bufs=2)
                    nc.vector.tensor_reduce(out=part[:], in_=cv,
                                            axis=AX.X, op=AluOpType.add)
                    nc.vector.tensor_add(out=csum[kb][:], in0=csum[kb][:],
                                         in1=part[:])
                if g == NB - 1:
                    # row 127 -> x3 src 1 ; finish x3 + P2 for this kb
                    xv = x3[kb][:].rearrange("p (s c) -> p s c", s=3)
                    nc.vector.tensor_copy(
                        out=xv[:, 1, 1:1 + W], in_=fch[ci][:, 15 * W:16 * W])
                    nc.vector.tensor_copy(out=xv[:, 0, 1:1 + W],
                                          in_=csum[kb][:])
                    cs2 = csum[kb][:].rearrange("p (w two) -> p w two", two=2)
                    nc.vector.tensor_tensor(
                        out=P2[kb][:], in0=cs2[:, :, 0], in1=cs2[:, :, 1],
                        op=AluOpType.add)

            # ================= helper: Part A for group g ==============
            WAv = WA_t[:].rearrange("p (kb sd m) -> p kb sd m", kb=NK, sd=9)
            WBv = WB_t[:].rearrange("p (kb cc m) -> p kb cc m", kb=NK, cc=16)
            WVv = WV_t[:].rearrange("p (kb sd m) -> p kb sd m", kb=NK, sd=9)
            Wqkv = Wqk_t[:].rearrange("p (kb s m) -> p kb s m", kb=NK, s=2)
            bv3 = [bev_t[kb][:].rearrange("p (h w) -> p h w", w=W)
                   for kb in range(NK)]
            stgA = [None] * NB

            def part_a(g):
                """Dense 3x3 conv over bev channels -> stgA[g] (fp16, +fb)."""
                stg = []
                for mb_i, (m0, ms) in enumerate(MBLK):
                    st = sap.tile([ms, 16 * W], F16, name=f"stgA{g}_{mb_i}",
                                  tag=f"stgA_{mb_i}", bufs=5)
                    stg.append(st)
                    for n in range(4):
                        pa = psa.tile([ms, 4 * W], F32, name="psAt", tag="psAt")
                        pav = pa[:].rearrange("p (r c) -> p r c", c=W)
                        # full-coverage tap first (dy=1, dx=1)
                        taps = [(1, 1)] + [(dy, dx) for dy in range(3)
                                           for dx in range(3)
                                           if not (dy == 1 and dx == 1)]
                        n_mm = 0
                        for dy, dx in taps:
                            r0 = 16 * g + 4 * n + dy - 1
                            rl, rh = 0, 4
                            if r0 < 0:
                                rl, r0 = 1, 0
                            if r0 + (4 - rl) > H:
                                rh = 3
                            nr = rh - rl
                            cl, ch = 0, W
                            if dx == 0:
                                cl = 1
                            if dx == 2:
                                ch = W - 1
                            ncol = ch - cl
                            for kb in range(NK):
                                rhs = bv3[kb][:, r0:r0 + nr,
                                              cl + dx - 1:cl + dx - 1 + ncol]
                                nc.tensor.matmul(
                                    pav[:, rl:rl + nr, cl:ch],
                                    WAv[:, kb, dy * 3 + dx, m0:m0 + ms],
                                    rhs,
                                    start=(n_mm == 0),
                                    stop=(n_mm == 2 * len(taps) - 1))
                                n_mm += 1
                        # evacuate on scalar engine: stgA = psum + fb (fp16)
                        nc.scalar.activation(
                            out=st[:, n * 4 * W:(n + 1) * 4 * W], in_=pa[:],
                            func=AF.Identity, bias=bias_ap["fb"][mb_i])
                stgA[g] = stg

            # ============ helper: Part B + combine for group g =========
            def part_b(g):
                tvs = [tp_t[kb][:].rearrange("p (h w) -> p h w", w=TPW)
                       for kb in range(NK)]
                gout = []
                for mb_i, (m0, ms) in enumerate(MBLK):
                    go = gop.tile([ms, 16 * W], F32, name=f"gout{mb_i}",
                                  tag=f"gout_{mb_i}", bufs=1)
                    gout.append(go)
                for ph in range(2):
                    ro = _tap_offsets(ph)
                    for pw in range(2):
                        co = _tap_offsets(pw)
                        for mb_i, (m0, ms) in enumerate(MBLK):
                            pb = psb.tile([ms, 8 * WP], F32, name="psBt",
                                          tag="psBt")
                            pbv = pb[:].rearrange("p (h w) -> p h w", w=WP)
                            # full-coverage tap (ro=0, co=0) first
                            ij = sorted(
                                ((i, j) for i in range(2) for j in range(2)),
                                key=lambda t: (ro[t[0]] != 0, co[t[1]] != 0))
                            n_mm = 0
                            for i, j in ij:
                                h0 = 8 * g + ro[i]
                                rl, rh = 0, 8
                                if h0 < 0:
                                    rl, h0 = 1, 0
                                if h0 + (8 - rl) > HP:
                                    rh = 7
                                nr = rh - rl
                                cc = ((ph * 2 + pw) * 2 + i) * 2 + j
                                for kb in range(NK):
                                    rhs = tvs[kb][:, h0:h0 + nr,
                                                  1 + co[j]:1 + co[j] + WP]
                                    nc.tensor.matmul(
                                        pbv[:, rl:rl + nr, :],
                                        WBv[:, kb, cc, m0:m0 + ms],
                                        rhs,
                                        start=(n_mm == 0),
                                        stop=(n_mm == 7))
                                    n_mm += 1
                            # combine on gpsimd: gout = psumB + stgA
                            gv = gout[mb_i][:].rearrange(
                                "p (h two w pw2) -> p h two w pw2",
                                two=2, w=WP, pw2=2)
                            sv = stgA[g][mb_i][:].rearrange(
                                "p (h two w pw2) -> p h two w pw2",
                                two=2, w=WP, pw2=2)
                            nc.vector.tensor_tensor(
                                out=gv[:, :, ph, :, pw], in0=pbv,
                                in1=sv[:, :, ph, :, pw], op=AluOpType.add)
                for mb_i, (m0, ms) in enumerate(MBLK):
                    nc.sync.dma_start(
                        out=out_d[16 + m0:16 + m0 + ms,
                                  16 * g:16 * (g + 1), :],
                        in_=gout[mb_i][:])

            # ================= prefix compute helpers ==================
            def prefix_vsum_k():
                # vsum: 1-D 3-tap convs over colsum with edge corrections
                for mb in range(2):
                    ps = psp.tile([128, W], F32, name="psS", tag="psP")
                    n_mm = 0
                    for sd in range(9):
                        src, dx = divmod(sd, 3)
                        for kb in range(NK):
                            xv = x3[kb][:].rearrange("p (s c) -> p s c", s=3)
                            nc.tensor.matmul(
                                ps[:], WVv[:, kb, sd, mb * 128:(mb + 1) * 128],
                                xv[:, src, dx:dx + W],
                                start=(n_mm == 0), stop=(n_mm == 17))
                            n_mm += 1
                    ssb = prp.tile([128, W], F32, name=f"ssb_{mb}",
                                   tag="ssb", bufs=2)
                    nc.scalar.activation(out=ssb[:], in_=ps[:], func=AF.Copy)
                    se = ssb[:].rearrange("p (w two) -> p w two", two=2)
                    nc.vector.scalar_tensor_tensor(
                        out=vsum_t[mb][:], in0=se[:, :, 0],
                        scalar=bias_ap["vb"][mb], in1=se[:, :, 1],
                        op0=AluOpType.add, op1=AluOpType.add)
                # k
                psk = psp.tile([C8, WP], F32, name="psK", tag="psP")
                nc.tensor.matmul(psk[:], Wqkv[:, 0, 1, :], P2[0][:],
                                 start=True, stop=False)
                nc.tensor.matmul(psk[:], Wqkv[:, 1, 1, :], P2[1][:],
                                 start=False, stop=True)
                nc.vector.tensor_scalar_add(out=k_t[:], in0=psk[:],
                                            scalar1=bias_ap["kb"])

            def prefix_q():
                qkv = qk_t[:].rearrange("p (h w) -> p h w", w=WP)
                for g in range(NB):
                    psq = psp.tile([C8, 8 * WP], F32, name="psQ", tag="psP")
                    n_mm = 0
                    for i in range(2):
                        for j in range(2):
                            for kb in range(NK):
                                rhs = bv3[kb][:, 16 * g + i:16 * g + 16:2,
                                              j:j + 127:2]
                                nc.tensor.matmul(
                                    psq[:], Wqkv[:, kb, 0, :], rhs,
                                    start=(n_mm == 0), stop=(n_mm == 7))
                                n_mm += 1
                    # qk = (psq + qb) * k  (fused, fp16 out)
                    kv = k_t[:].unsqueeze(1).broadcast_to([C8, 8, WP])
                    nc.vector.scalar_tensor_tensor(
                        out=qkv[:, g * 8:(g + 1) * 8, :],
                        in0=psq[:].rearrange("p (h w) -> p h w", w=WP),
                        scalar=bias_ap["qb"], in1=kv,
                        op0=AluOpType.add, op1=AluOpType.mult)

            def prefix_e_tp():
                # e replicated across 128 partitions via ones matmul
                for chn in range(8):
                    nsl = slice(chn * 512, (chn + 1) * 512)
                    pse = psp.tile([128, 512], F32, name="psE", tag="psP")
                    nc.tensor.matmul(pse[:], ones_t[:], qk_t[:, nsl],
                                     start=True, stop=True)
                    nc.scalar.activation(out=e_t[:, nsl], in_=pse[:],
                                         func=AF.Copy)
                # column norms: n2[w] = sum_h e^2 (gpsimd)
                ev = e_t[:].rearrange("p (h w) -> p h w", w=WP)
                for chn in range(8):
                    scr = prp.tile([128, 8 * WP], F32, name="scr",
                                   tag="scr", bufs=2)
                    esl = e_t[:, chn * 8 * WP:(chn + 1) * 8 * WP]
                    nc.gpsimd.tensor_tensor(out=scr[:], in0=esl, in1=esl,
                                            op=AluOpType.mult)
                    part = prp.tile([128, WP], F32, name="npart",
                                    tag="npart", bufs=2)
                    nc.vector.tensor_reduce(
                        out=part[:],
                        in_=scr[:].rearrange("p (h w) -> p w h", w=WP),
                        axis=AX.X, op=AluOpType.add)
                    if chn == 0:
                        nc.vector.tensor_copy(out=n2[:], in_=part[:])
                    else:
                        nc.vector.tensor_add(out=n2[:], in0=n2[:],
                                             in1=part[:])
                nc.scalar.sqrt(out=nrm[:], in_=n2[:])
                nc.vector.reciprocal(out=rinv[:], in_=nrm[:])
                for kb in range(NK):
                    nc.vector.tensor_tensor(
                        out=vs2[kb][:], in0=vsum_t[kb][:], in1=rinv[:],
                        op=AluOpType.mult)
                    tv = tp_t[kb][:].rearrange("p (h w) -> p h w", w=TPW)
                    v2 = vs2[kb][:].unsqueeze(1).broadcast_to([128, HP, WP])
                    nc.vector.tensor_tensor(
                        out=tv[:, :, 1:1 + WP], in0=v2, in1=ev,
                        op=AluOpType.mult)

            # ======================= schedule ==========================
            part_a(0)
            part_a(1)
            prefix_vsum_k()
            part_a(2)
            prefix_q()
            part_a(3)
            prefix_e_tp()
            part_a(4)
            part_b(0)
            part_a(5)
            part_b(1)
            part_a(6)
            part_b(2)
            part_a(7)
            for g in range(3, NB):
                part_b(g)
    return nc


def _legalize_waits(nc):
    """This toolchain's codegen accepts at most ONE semaphore wait per
    instruction (the TPB `events` field has a single wait slot). Tile's
    wait assignment can attach several. Hoist all but one wait onto
    standalone EventSemaphore instructions placed immediately before the
    owner on the same engine stream - strictly stronger synchronization,
    so always safe."""
    n_split = 0
    for fn in nc.m.functions:
        for bb in fn.blocks:
            out = []
            for ins in bb.instructions:
                si = ins.sync_info
                if si is not None and len(si.on_wait) > 1:
                    extra = list(si.on_wait[:-1])
                    keep = si.on_wait[-1]
                    for idx, wt in enumerate(extra):
                        ev = mybir.InstEventSemaphore(
                            name=f"{ins.name}_hw{idx}",
                            engine=ins.engine,
                            sync_info=mybir.SyncInfo(on_wait=[wt],
                                                     on_update=[]),
                        )
                        out.append(ev)
                        n_split += 1
                    ins.sync_info = mybir.SyncInfo(
                        on_wait=[keep], on_update=list(si.on_update))
                out.append(ins)
            bb.instructions[:] = out
    return n_split


_NC_CACHE = None


def kernel(**inputs):
    global _NC_CACHE
    in_maps = _prep_inputs(inputs)
    if _NC_CACHE is None:
        _NC_CACHE = _build_module()
        _legalize_waits(_NC_CACHE)
    res = run_bass_kernel_spmd(_NC_CACHE, in_maps, list(range(NCORES)))
    out = np.stack([res.results[b]["out"] for b in range(NCORES)], axis=0)
    return out.astype(np.float32)
